# revision 15
# baseline (speedup 1.0000x reference)
"""ARMA cell kernel for Trainium2, 8 NeuronCores.

Computation (B=1024, K=256, UNITS=32, P=Q=4):
  ar[b,j,u] = sum_{i,p} inputs[b,i,p] * kernel[p,u,i,j]
  ma[b,j,u] = sum_{i,q} state[b,i*32+u,q] * rkernel[q,u,i,j]
  out       = (ar+ma).reshape(B, K*UNITS)[:, :, None]
  out_state = concat([out, state[:, :, :-1]], axis=-1)

Sharding: units are split across the 8 cores (4 units/core); each core sees
the full batch.  Per unit the contraction is 8 groups (4 AR "p" lags + 4 MA
"q" lags) x 2 chunks of 128 over i.  TensorE computes
  psum[jp, b] += wt[ip, jp].T @ act[ip, b]
with the weight chunk stationary and a 512-wide batch slice moving;
accumulation is fp32 in PSUM.

COMPUTE_DT picks the PE input dtype: "bf16" halves DMA bytes (the kernel is
DMA-bound at fp32), "f32r" is full-rate fp32 storage with ~1e-4 rel err.

Host side pre-transposes inputs/state/weights into DMA-friendly layouts
(contraction index on the partition dim, every dma_start fully contiguous)
and assembles the outputs.
"""

import os
import sys

import numpy as np

try:
    import concourse.bass as bass
except ImportError:  # fresh grading dir: put the container repos on sys.path
    for p in ("/opt/trn_rl_repo", "/opt/pypackages"):
        if p not in sys.path:
            sys.path.insert(0, p)
    import concourse.bass as bass

import ml_dtypes
import concourse.mybir as mybir
import concourse.tile as tile
from concourse import bacc, bass_utils

B = 1024
K = 256
UNITS = 32
P = 4
Q = 4
NCORES = 8
UPC = UNITS // NCORES  # units per core
IC = K // 128          # contraction chunks over i
JC = K // 128          # output chunks over j
G = P + Q              # contraction groups (AR lags + MA lags)
BF = 512               # moving free dim per matmul (one fp32 PSUM bank)
BC = B // BF

F32 = mybir.dt.float32

COMPUTE_DT = os.environ.get("ARMA_COMPUTE_DT", "bf16")  # "bf16" | "f32r"
if COMPUTE_DT == "bf16":
    DT_IN = mybir.dt.bfloat16
    NP_IN = np.dtype(ml_dtypes.bfloat16)
    DT_OUT = mybir.dt.bfloat16
else:
    DT_IN = mybir.dt.float32r
    NP_IN = np.dtype(np.float32)
    DT_OUT = mybir.dt.float32

LAST_RESULT = None  # BassKernelResults of the most recent run (for test.py)


def _build_bass():
    nc = bacc.Bacc("TRN2", target_bir_lowering=False)
    # all layouts chosen so every dma_start reads/writes contiguous DRAM
    xt = nc.declare_dram_parameter("xt", [BC, IC, 128, P, BF], DT_IN, isOutput=False)
    st = nc.declare_dram_parameter("st", [UPC, BC, IC, 128, Q, BF], DT_IN, isOutput=False)
    wt = nc.declare_dram_parameter("wt", [UPC, 128, G, IC, K], DT_IN, isOutput=False)
    out = nc.declare_dram_parameter("out", [UPC, JC, BC, 128, BF], DT_OUT, isOutput=True)

    with tile.TileContext(nc) as tc:
        with (
            tc.tile_pool(name="const", bufs=1) as cpool,
            tc.tile_pool(name="state", bufs=2) as spool,
            tc.tile_pool(name="wts", bufs=2) as wpool,
            tc.tile_pool(name="outs", bufs=4) as opool,
            tc.tile_pool(name="ps", bufs=6, space="PSUM") as pspool,
        ):
            xt_sb = cpool.tile([128, IC, P, B], DT_IN)
            wt_sbs = []
            st_sbs = []

            def dma_xt(h):
                for ic in range(IC):
                    nc.sync.dma_start(
                        out=xt_sb[:, ic, :, h * BF : (h + 1) * BF],
                        in_=xt[h, ic],
                    )

            def dma_unit(u, h):
                for ic in range(IC):
                    nc.sync.dma_start(
                        out=st_sbs[u][:, ic, :, h * BF : (h + 1) * BF],
                        in_=st[u, h, ic],
                    )

            def alloc_unit(u):
                assert len(wt_sbs) == u
                wt_sbs.append(wpool.tile([128, G, IC, K], DT_IN, tag="wt", name=f"wt_sb{u}"))
                st_sbs.append(spool.tile([128, IC, Q, B], DT_IN, tag="st", name=f"st_sb{u}"))

            # startup ordering: stream unit 0 in the order the first psum
            # group's matmuls consume it (per-g weight slices, fine-grained);
            # the first deps ride the scalar HWDGE ring, whose engine preamble
            # finishes ~1us before sync's
            alloc_unit(0)
            nc.scalar.dma_start(out=wt_sbs[0][:, 0], in_=wt[0, :, 0])
            for ic in range(IC):
                nc.scalar.dma_start(
                    out=xt_sb[:, ic, :, 0:BF],
                    in_=xt[0, ic],
                )
            for g in range(1, P):
                nc.sync.dma_start(out=wt_sbs[0][:, g], in_=wt[0, :, g])
            dma_unit(0, 0)
            for g in range(P, G):
                nc.sync.dma_start(out=wt_sbs[0][:, g], in_=wt[0, :, g])
            dma_xt(1)
            dma_unit(0, 1)

            for u in range(UPC):
                if u > 0:
                    alloc_unit(u)
                    nc.sync.dma_start(out=wt_sbs[u], in_=wt[u])
                    for h in range(BC):
                        dma_unit(u, h)
                wt_sb, st_sb = wt_sbs[u], st_sbs[u]

                for bc in range(BC):
                    bsl = slice(bc * BF, (bc + 1) * BF)
                    for jc in range(JC):
                        ps = pspool.tile([128, BF], F32, tag="ps")
                        n = 0
                        for g in range(G):
                            for ic in range(IC):
                                lhsT = wt_sb[:, g, ic, jc * 128 : (jc + 1) * 128]
                                if g < P:
                                    rhs = xt_sb[:, ic, g, bsl]
                                else:
                                    rhs = st_sb[:, ic, g - P, bsl]
                                nc.tensor.matmul(
                                    ps,
                                    lhsT,
                                    rhs,
                                    start=(n == 0),
                                    stop=(n == G * IC - 1),
                                )
                                n += 1
                        ot = opool.tile([128, BF], DT_OUT, tag="ot")
                        last = (u == UPC - 1) and (bc == BC - 1) and (jc == JC - 1)
                        if last:
                            # pipeline the tail: DMA each half right after its copy
                            for hh in range(2):
                                hsl = slice(hh * (BF // 2), (hh + 1) * (BF // 2))
                                nc.vector.tensor_copy(out=ot[:, hsl], in_=ps[:, hsl])
                                nc.scalar.dma_start(
                                    out=out[u, jc, bc, :, hsl], in_=ot[:, hsl]
                                )
                        else:
                            nc.vector.tensor_copy(out=ot, in_=ps)
                            # separate HWDGE ring (qActDynamicHW) for stores
                            nc.scalar.dma_start(out=out[u, jc, bc], in_=ot)
    nc.compile()
    return nc


def _prep_inputs(inputs, state, kernel, recurrent_kernel):
    """Host-side resharding into the DMA layouts declared in _build_bass."""
    inputs = np.ascontiguousarray(inputs, dtype=np.float32)
    state = np.ascontiguousarray(state, dtype=np.float32)
    kernel = np.ascontiguousarray(kernel, dtype=np.float32)
    recurrent_kernel = np.ascontiguousarray(recurrent_kernel, dtype=np.float32)

    # xt[h, ic, ip, p, bf] = inputs[h*BF+bf, ic*128+ip, p]
    xt = np.ascontiguousarray(
        inputs.reshape(BC, BF, IC, 128, P).transpose(0, 2, 3, 4, 1), dtype=NP_IN
    )

    # st_all[u, h, ic, ip, q, bf] = state[h*BF+bf, (ic*128+ip)*UNITS + u, q]
    st_all = np.ascontiguousarray(
        state.reshape(BC, BF, IC, 128, UNITS, Q).transpose(4, 0, 2, 3, 5, 1),
        dtype=NP_IN,
    )

    # wt_all[u, ip, g, ic, j]: g<P -> kernel[g,u,...], else rkernel[g-P,u,...]
    wk = kernel.reshape(P, UNITS, IC, 128, K)
    wr = recurrent_kernel.reshape(Q, UNITS, IC, 128, K)
    wg = np.concatenate([wk, wr], axis=0)  # (G, UNITS, IC, 128, K)
    wt_all = np.ascontiguousarray(wg.transpose(1, 3, 0, 2, 4), dtype=NP_IN)

    in_maps = []
    for c in range(NCORES):
        usl = slice(c * UPC, (c + 1) * UPC)
        in_maps.append(
            {
                "xt": xt,
                "st": np.ascontiguousarray(st_all[usl]),
                "wt": np.ascontiguousarray(wt_all[usl]),
            }
        )
    return in_maps


def kernel(inputs, state, kernel, recurrent_kernel):
    global LAST_RESULT
    state = np.asarray(state, dtype=np.float32)
    in_maps = _prep_inputs(inputs, state, kernel, recurrent_kernel)
    nc = _build_bass()
    res = bass_utils.run_bass_kernel_spmd(nc, in_maps, core_ids=list(range(NCORES)))
    LAST_RESULT = res

    # results[c]["out"][u, jc, bc, jp, bf] -> out[bc*BF+bf, (jc*128+jp)*UNITS + c*UPC+u]
    outs = np.stack(
        [np.asarray(r["out"], dtype=np.float32) for r in res.results]
    )  # (8, UPC, JC, BC, 128, BF)
    out = np.ascontiguousarray(outs.transpose(3, 5, 2, 4, 0, 1)).reshape(B, K * UNITS)
    out = out[:, :, None]
    out_state = np.concatenate([out, state[:, :, :-1]], axis=-1)
    return out, out_state


# revision 16
# speedup vs baseline: 1.0357x; 1.0357x over previous
"""ARMA cell kernel for Trainium2, 8 NeuronCores.

Computation (B=1024, K=256, UNITS=32, P=Q=4):
  ar[b,j,u] = sum_{i,p} inputs[b,i,p] * kernel[p,u,i,j]
  ma[b,j,u] = sum_{i,q} state[b,i*32+u,q] * rkernel[q,u,i,j]
  out       = (ar+ma).reshape(B, K*UNITS)[:, :, None]
  out_state = concat([out, state[:, :, :-1]], axis=-1)

Sharding: units are split across the 8 cores (4 units/core); each core sees
the full batch.  Per unit the contraction is 8 groups (4 AR "p" lags + 4 MA
"q" lags) x 2 chunks of 128 over i.  TensorE computes
  psum[jp, b] += wt[ip, jp].T @ act[ip, b]
with the weight chunk stationary and a 512-wide batch slice moving;
accumulation is fp32 in PSUM.

COMPUTE_DT picks the PE input dtype: "bf16" halves DMA bytes (the kernel is
DMA-bound at fp32), "f32r" is full-rate fp32 storage with ~1e-4 rel err.

Host side pre-transposes inputs/state/weights into DMA-friendly layouts
(contraction index on the partition dim, every dma_start fully contiguous)
and assembles the outputs.
"""

import os
import sys

import numpy as np

try:
    import concourse.bass as bass
except ImportError:  # fresh grading dir: put the container repos on sys.path
    for p in ("/opt/trn_rl_repo", "/opt/pypackages"):
        if p not in sys.path:
            sys.path.insert(0, p)
    import concourse.bass as bass

import ml_dtypes
import concourse.mybir as mybir
import concourse.tile as tile
from concourse import bacc, bass_utils

B = 1024
K = 256
UNITS = 32
P = 4
Q = 4
NCORES = 8
UPC = UNITS // NCORES  # units per core
IC = K // 128          # contraction chunks over i
JC = K // 128          # output chunks over j
G = P + Q              # contraction groups (AR lags + MA lags)
BF = 512               # moving free dim per matmul (one fp32 PSUM bank)
BC = B // BF

F32 = mybir.dt.float32

COMPUTE_DT = os.environ.get("ARMA_COMPUTE_DT", "bf16")  # "bf16" | "f32r"
if COMPUTE_DT == "bf16":
    DT_IN = mybir.dt.bfloat16
    NP_IN = np.dtype(ml_dtypes.bfloat16)
    DT_OUT = mybir.dt.bfloat16
else:
    DT_IN = mybir.dt.float32r
    NP_IN = np.dtype(np.float32)
    DT_OUT = mybir.dt.float32

LAST_RESULT = None  # BassKernelResults of the most recent run (for test.py)


def _build_bass():
    nc = bacc.Bacc("TRN2", target_bir_lowering=False)
    # all layouts chosen so every dma_start reads/writes contiguous DRAM
    xt = nc.declare_dram_parameter("xt", [BC, IC, 128, P, BF], DT_IN, isOutput=False)
    st = nc.declare_dram_parameter("st", [UPC, BC, IC, 128, Q, BF], DT_IN, isOutput=False)
    wt = nc.declare_dram_parameter("wt", [UPC, 128, G, IC, K], DT_IN, isOutput=False)
    out = nc.declare_dram_parameter("out", [UPC, JC, BC, 128, BF], DT_OUT, isOutput=True)

    with tile.TileContext(nc) as tc:
        with (
            tc.tile_pool(name="const", bufs=1) as cpool,
            tc.tile_pool(name="state", bufs=2) as spool,
            tc.tile_pool(name="wts", bufs=2) as wpool,
            tc.tile_pool(name="outs", bufs=4) as opool,
            tc.tile_pool(name="ps", bufs=4, space="PSUM") as pspool,
        ):
            xt_sb = cpool.tile([128, IC, P, B], DT_IN)
            wt_sbs = []
            st_sbs = []

            def dma_xt(h):
                for ic in range(IC):
                    nc.sync.dma_start(
                        out=xt_sb[:, ic, :, h * BF : (h + 1) * BF],
                        in_=xt[h, ic],
                    )

            def dma_unit(u, h):
                for ic in range(IC):
                    nc.sync.dma_start(
                        out=st_sbs[u][:, ic, :, h * BF : (h + 1) * BF],
                        in_=st[u, h, ic],
                    )

            def alloc_unit(u):
                assert len(wt_sbs) == u
                wt_sbs.append(wpool.tile([128, G, IC, K], DT_IN, tag="wt", name=f"wt_sb{u}"))
                st_sbs.append(spool.tile([128, IC, Q, B], DT_IN, tag="st", name=f"st_sb{u}"))

            # startup ordering: stream unit 0 in the order the first psum
            # group's matmuls consume it (per-g weight slices, fine-grained)
            alloc_unit(0)
            nc.sync.dma_start(out=wt_sbs[0][:, 0], in_=wt[0, :, 0])
            dma_xt(0)
            for g in range(1, P):
                nc.sync.dma_start(out=wt_sbs[0][:, g], in_=wt[0, :, g])
            dma_unit(0, 0)
            for g in range(P, G):
                nc.sync.dma_start(out=wt_sbs[0][:, g], in_=wt[0, :, g])
            dma_xt(1)
            dma_unit(0, 1)

            for u in range(UPC):
                if u > 0:
                    alloc_unit(u)
                    nc.sync.dma_start(out=wt_sbs[u], in_=wt[u])
                    for h in range(BC):
                        dma_unit(u, h)
                wt_sb, st_sb = wt_sbs[u], st_sbs[u]

                for bc in range(BC):
                    bsl = slice(bc * BF, (bc + 1) * BF)
                    for jc in range(JC):
                        ps = pspool.tile([128, BF], F32, tag="ps")
                        n = 0
                        for g in range(G):
                            for ic in range(IC):
                                lhsT = wt_sb[:, g, ic, jc * 128 : (jc + 1) * 128]
                                if g < P:
                                    rhs = xt_sb[:, ic, g, bsl]
                                else:
                                    rhs = st_sb[:, ic, g - P, bsl]
                                nc.tensor.matmul(
                                    ps,
                                    lhsT,
                                    rhs,
                                    start=(n == 0),
                                    stop=(n == G * IC - 1),
                                )
                                n += 1
                        ot = opool.tile([128, BF], DT_OUT, tag="ot")
                        last = (u == UPC - 1) and (bc == BC - 1) and (jc == JC - 1)
                        if last:
                            # pipeline the tail: DMA each half right after its copy
                            for hh in range(2):
                                hsl = slice(hh * (BF // 2), (hh + 1) * (BF // 2))
                                nc.vector.tensor_copy(out=ot[:, hsl], in_=ps[:, hsl])
                                nc.scalar.dma_start(
                                    out=out[u, jc, bc, :, hsl], in_=ot[:, hsl]
                                )
                        else:
                            nc.vector.tensor_copy(out=ot, in_=ps)
                            # separate HWDGE ring (qActDynamicHW) for stores
                            nc.scalar.dma_start(out=out[u, jc, bc], in_=ot)
    nc.compile()
    return nc


def _prep_inputs(inputs, state, kernel, recurrent_kernel):
    """Host-side resharding into the DMA layouts declared in _build_bass."""
    inputs = np.ascontiguousarray(inputs, dtype=np.float32)
    state = np.ascontiguousarray(state, dtype=np.float32)
    kernel = np.ascontiguousarray(kernel, dtype=np.float32)
    recurrent_kernel = np.ascontiguousarray(recurrent_kernel, dtype=np.float32)

    # xt[h, ic, ip, p, bf] = inputs[h*BF+bf, ic*128+ip, p]
    xt = np.ascontiguousarray(
        inputs.reshape(BC, BF, IC, 128, P).transpose(0, 2, 3, 4, 1), dtype=NP_IN
    )

    # st_all[u, h, ic, ip, q, bf] = state[h*BF+bf, (ic*128+ip)*UNITS + u, q]
    st_all = np.ascontiguousarray(
        state.reshape(BC, BF, IC, 128, UNITS, Q).transpose(4, 0, 2, 3, 5, 1),
        dtype=NP_IN,
    )

    # wt_all[u, ip, g, ic, j]: g<P -> kernel[g,u,...], else rkernel[g-P,u,...]
    wk = kernel.reshape(P, UNITS, IC, 128, K)
    wr = recurrent_kernel.reshape(Q, UNITS, IC, 128, K)
    wg = np.concatenate([wk, wr], axis=0)  # (G, UNITS, IC, 128, K)
    wt_all = np.ascontiguousarray(wg.transpose(1, 3, 0, 2, 4), dtype=NP_IN)

    in_maps = []
    for c in range(NCORES):
        usl = slice(c * UPC, (c + 1) * UPC)
        in_maps.append(
            {
                "xt": xt,
                "st": np.ascontiguousarray(st_all[usl]),
                "wt": np.ascontiguousarray(wt_all[usl]),
            }
        )
    return in_maps


def kernel(inputs, state, kernel, recurrent_kernel):
    global LAST_RESULT
    state = np.asarray(state, dtype=np.float32)
    in_maps = _prep_inputs(inputs, state, kernel, recurrent_kernel)
    nc = _build_bass()
    res = bass_utils.run_bass_kernel_spmd(nc, in_maps, core_ids=list(range(NCORES)))
    LAST_RESULT = res

    # results[c]["out"][u, jc, bc, jp, bf] -> out[bc*BF+bf, (jc*128+jp)*UNITS + c*UPC+u]
    outs = np.stack(
        [np.asarray(r["out"], dtype=np.float32) for r in res.results]
    )  # (8, UPC, JC, BC, 128, BF)
    out = np.ascontiguousarray(outs.transpose(3, 5, 2, 4, 0, 1)).reshape(B, K * UNITS)
    out = out[:, :, None]
    out_state = np.concatenate([out, state[:, :, :-1]], axis=-1)
    return out, out_state
